# revision 26
# baseline (speedup 1.0000x reference)
"""MixerDiffAttention Trainium2 kernel (v3).

Sharding: 8 cores = batch(2) x head-group(4).  Core (b, r) computes output
heads {2r, 2r+1} of batch b: q-heads {2r,2r+1,8+2r,8+2r+1}, k-heads {r, 4+r},
v-head r.

Design:
 - Act engine runs ONLY Exp (zero activation-table reloads).
 - fp16 q/k pipeline (host-cast inputs), bf16 pt/vA for PV (bf16 range
   needed for max-free exp).  Scores fp16 x fp16 -> f32 PSUM.
 - Transposes via batched DMA xbar: one [128,512]->[128,4,128] per tile for
   q (sync queue) and one [128,256]->[128,2,128] for k (scalar queue).
 - Causal diag mask pre-loaded into PSUM by an extra matmul in the same
   accumulation group (no vector mask work).
 - RMS rsqrt via bit-trick + Newton on DVE; sums of squares via DVE
   mul+reduce; PSUM drains on DVE; SBUF-only elementwise (rope-k, final
   gn scale) on Pool.
 - Schedule: proj(c+2) + rope tiles interleaved into attn(c) segments so
   the PE and Act never wait on bursts.
"""
import os
import numpy as np
import concourse.bass as bass
import concourse.mybir as mybir
from concourse import bacc
from concourse.tile import TileContext
from concourse.bass_utils import run_bass_kernel_spmd

B, T, DM = 2, 2048, 2048
H, KVH, D = 16, 8, 128
TC = 512                  # token chunk (= q chunk)
NT = T // 128             # 16 token tiles
NCH = T // TC             # 4 chunks
NDM = DM // 128           # 16 contraction chunks
EPS = 1e-6
ROPE_BASE = 10000.0
LAMBDA_INIT = 0.8 - 0.6 * np.exp(-0.3 * 12)
F32 = mybir.dt.float32
FP16 = mybir.dt.float16
BF16 = mybir.dt.bfloat16
I32 = mybir.dt.int32
AF = mybir.ActivationFunctionType
ALU = mybir.AluOpType
AX = mybir.AxisListType
ISQ = float(1.0 / np.sqrt(D))
MASK_NEG = -60000.0       # fp16-safe; exp(ISQ*(s+MASK_NEG)) == 0 in f32
RSQRT_MAGIC2 = 0x5EF759DF   # seed magic for msh = ms/2 inputs


def _bc_mid(a, n):
    # [128, m] AP -> [128, n(bcast), m]
    return bass.AP(tensor=a.tensor, offset=a.offset, ap=[list(a.ap[0]), [0, n], list(a.ap[1])])


def _bc_last(a, n):
    # [128, m] AP -> [128, m, n(bcast)]
    return bass.AP(tensor=a.tensor, offset=a.offset, ap=[list(a.ap[0]), list(a.ap[1]), [0, n]])


def _build():
    nc = bacc.Bacc(None, target_bir_lowering=False)

    xT = nc.dram_tensor("xT", [DM, T], FP16, kind="ExternalInput")
    wq_d = nc.dram_tensor("wq", [DM, 512], FP16, kind="ExternalInput")
    wkv_d = nc.dram_tensor("wkv", [DM, 512], FP16, kind="ExternalInput")
    lsc_d = nc.dram_tensor("lsc", [128, NT, 4], F32, kind="ExternalInput")
    gn_d = nc.dram_tensor("gnw", [128, 2, 256], F32, kind="ExternalInput")
    neglam_d = nc.dram_tensor("neglam", [128, 1], F32, kind="ExternalInput")
    y_d = nc.dram_tensor("y", [T, 2, 256], F32, kind="ExternalOutput")

    # constant tables, laid out [128 partitions, NT tiles, ...] host-side
    pos = np.arange(T, dtype=np.float64)
    inv = ROPE_BASE ** (-np.arange(0, D, 2, dtype=np.float64) / D)  # (64,)
    ang = np.outer(pos, inv)                                       # (T, 64)
    cos_t = np.cos(ang).reshape(NT, 128, 64)
    sin_t = np.sin(ang).reshape(NT, 128, 64)
    cosf = np.concatenate([cos_t, cos_t], axis=2)          # (NT,128,128)
    sinf = np.concatenate([sin_t, -sin_t], axis=2)
    cosf4 = np.broadcast_to(cosf[:, :, None, :], (NT, 128, 4, 128))
    sinf4 = np.broadcast_to(sinf[:, :, None, :], (NT, 128, 4, 128))
    cosf4_h = cosf4.transpose(1, 0, 2, 3).astype(np.float16).copy()
    sinf4_h = sinf4.transpose(1, 0, 2, 3).astype(np.float16).copy()
    pidx = np.arange(128).reshape(128, 1)
    fidx = np.arange(512).reshape(1, 512)
    triw_h = np.where((pidx > fidx) & (fidx < 128), np.float16(MASK_NEG),
                      np.float16(0.0)).astype(np.float16)       # [128, 512]
    ident_h = np.eye(128, dtype=np.float16)

    cos_c = nc.inline_tensor(cosf4_h, "cos_c")
    sin_c = nc.inline_tensor(sinf4_h, "sin_c")
    triw_c = nc.inline_tensor(triw_h, "triw_c")
    ident_c = nc.inline_tensor(ident_h, "ident_c")

    with TileContext(nc) as tc:
        with (
            tc.tile_pool(name="wp", bufs=1) as wp,
            tc.tile_pool(name="cp", bufs=1) as cp,
            tc.tile_pool(name="xp", bufs=34) as xp,
            tc.tile_pool(name="kv", bufs=1) as kvp,
            tc.tile_pool(name="qt", bufs=2) as qtp,
            tc.tile_pool(name="wk", bufs=2) as wk,
            tc.tile_pool(name="qh", bufs=6) as qhp,
            tc.tile_pool(name="qr", bufs=4) as qrp,
            tc.tile_pool(name="pt", bufs=18) as ptp,
            tc.tile_pool(name="yv", bufs=5) as yvp,
            tc.tile_pool(name="yo", bufs=4) as yop,
            tc.tile_pool(name="ps_p", bufs=2, space="PSUM") as ps_p,
            tc.tile_pool(name="ps_s", bufs=4, space="PSUM") as ps_s,
            tc.tile_pool(name="ps_o", bufs=2, space="PSUM") as ps_o,
        ):
            # ---- persistent loads (spread across queues; wq + x(0) first) ----
            wq_sb = wp.tile([128, NDM, 512], FP16, tag="wq")
            wkv_sb = wp.tile([128, NDM, 512], FP16, tag="wkv")
            nc.sync.dma_start(out=wq_sb, in_=wq_d.ap().rearrange("(n p) m -> p n m", p=128))

            xts = {}

            def emit_x_load(c):
                tiles = []
                for dmi in range(NDM):
                    xt_t = xp.tile([128, TC], FP16, tag="xt")
                    nc.scalar.dma_start(
                        out=xt_t,
                        in_=xT.ap()[dmi * 128:(dmi + 1) * 128, c * TC:(c + 1) * TC],
                    )
                    tiles.append(xt_t)
                xts[c] = tiles

            emit_x_load(0)
            nc.sync.dma_start(out=wkv_sb, in_=wkv_d.ap().rearrange("(n p) m -> p n m", p=128))
            emit_x_load(1)

            cos_sb = cp.tile([128, NT, 4, 128], FP16, tag="cos")
            sin_sb = cp.tile([128, NT, 4, 128], FP16, tag="sin")
            lsc_sb = cp.tile([128, NT, 4], F32, tag="lsc")
            rskA = cp.tile([128, NT, 2], F32, tag="rskA")
            c15_sb = cp.tile([128, 24], F32, tag="c15")
            triw_sb = cp.tile([128, 512], FP16, tag="triw")
            ident_sb = cp.tile([128, 128], FP16, tag="ident")
            gn_sb = cp.tile([128, 2, 256], F32, tag="gn")
            neglam_sb = cp.tile([128, 1], F32, tag="neglam")
            magic_sb = cp.tile([128, 24], I32, tag="magic")
            nc.scalar.dma_start(out=cos_sb, in_=cos_c.ap())
            nc.scalar.dma_start(out=sin_sb, in_=sin_c.ap())
            nc.scalar.dma_start(out=lsc_sb, in_=lsc_d.ap())
            nc.scalar.dma_start(out=triw_sb, in_=triw_c.ap())
            nc.scalar.dma_start(out=ident_sb, in_=ident_c.ap())
            nc.scalar.dma_start(out=gn_sb, in_=gn_d.ap())
            nc.scalar.dma_start(out=neglam_sb, in_=neglam_d.ap())
            nc.vector.memset(magic_sb[:], RSQRT_MAGIC2)
            nc.vector.memset(c15_sb[:], 1.5)

            # per-token-tile persistent K^T and V(+ones)
            kT_t = [kvp.tile([128, 2, 128], FP16, tag=f"kT{i}", name=f"kT{i}") for i in range(NT)]
            vA_t = [kvp.tile([128, 258], BF16, tag=f"vA{i}", name=f"vA{i}") for i in range(NT)]
            for i in range(NT):
                nc.gpsimd.memset(vA_t[i][:, 256:258], 1.0)

            ssq_ch = {}   # [128, 4, 6] f32 per chunk
            qh_ch = {}    # list of (q_h, k_h) per chunk
            rs_ch = {}    # (rsq_h [128,4,4] fp16, rsk_h [128,4,2] fp16)
            qT_ch = {}    # [128, 4, TC] fp16 per chunk (rotating pool bufs=2)

            def proj_tile(c, ti):
                tt = c * 4 + ti
                q_ps = ps_p.tile([128, 512], F32, tag="pp")
                kv_ps = ps_p.tile([128, 512], F32, tag="pp")
                for dmi in range(NDM):
                    lhs = xts[c][dmi][:, ti * 128:(ti + 1) * 128]
                    nc.tensor.matmul(q_ps[:], lhs, wq_sb[:, dmi, :],
                                     start=(dmi == 0), stop=(dmi == NDM - 1))
                for dmi in range(NDM):
                    lhs = xts[c][dmi][:, ti * 128:(ti + 1) * 128]
                    nc.tensor.matmul(kv_ps[:], lhs, wkv_sb[:, dmi, :],
                                     start=(dmi == 0), stop=(dmi == NDM - 1))
                # drain PSUM on DVE (only engine allowed); squares via mul+reduce
                q_h = qhp.tile([128, 512], FP16, tag="qh")
                k_h = qhp.tile([128, 256], FP16, tag="kh")
                nc.vector.tensor_copy(out=q_h[:], in_=q_ps[:])
                nc.vector.tensor_copy(out=k_h[:], in_=kv_ps[:, 0:256])
                nc.vector.tensor_copy(out=vA_t[tt][:, 0:256], in_=kv_ps[:, 256:512])
                if ti == 0:
                    ssq_ch[c] = wk.tile([128, 4, 6], F32, tag="ssq", name=f"ssq{c}")
                    qh_ch[c] = []
                qh_ch[c].append((q_h, k_h))
                sqv = wk.tile([128, 4, 128], FP16, tag="sqv")
                nc.vector.tensor_mul(sqv[:], q_h[:].rearrange("p (h d) -> p h d", h=4),
                                     q_h[:].rearrange("p (h d) -> p h d", h=4))
                nc.vector.tensor_reduce(ssq_ch[c][:, ti, 0:4], sqv[:], axis=AX.X, op=ALU.add)
                skv = wk.tile([128, 2, 128], FP16, tag="skv")
                nc.vector.tensor_mul(skv[:], k_h[:].rearrange("p (h d) -> p h d", h=2),
                                     k_h[:].rearrange("p (h d) -> p h d", h=2))
                nc.vector.tensor_reduce(ssq_ch[c][:, ti, 4:6], skv[:], axis=AX.X, op=ALU.add)

            def newton_rsqrt(msh_ap, n, tag, iters=2, eng=None):
                # rsqrt(2*msh): bit-trick seed on DVE, Newton iters on Pool
                # y' = y*(1.5 - msh*y*y)
                sh = wk.tile([128, n], I32, tag=tag + "sh")
                nc.vector.tensor_scalar(out=sh[:], in0=msh_ap.bitcast(I32), scalar1=1,
                                        scalar2=None, op0=ALU.logical_shift_right)
                y0 = wk.tile([128, n], F32, tag=tag + "y0")
                nc.vector.tensor_tensor(out=y0[:].bitcast(I32), in0=magic_sb[:, 0:n],
                                        in1=sh[:], op=ALU.subtract)
                a = wk.tile([128, n], F32, tag=tag + "a")
                e = eng if eng is not None else nc.gpsimd
                for _ in range(iters):
                    e.tensor_tensor(out=a[:], in0=msh_ap, in1=y0[:], op=ALU.mult)
                    e.tensor_tensor(out=a[:], in0=a[:], in1=y0[:], op=ALU.mult)
                    e.tensor_tensor(out=a[:], in0=c15_sb[:, 0:n], in1=a[:],
                                    op=ALU.subtract)
                    e.tensor_tensor(out=y0[:], in0=y0[:], in1=a[:], op=ALU.mult)
                return y0

            def newton_pair(c, p):
                # tiles 2p, 2p+1 of chunk c: rsqrt of q/k mean-squares
                tt0 = 4 * c + 2 * p
                ms = wk.tile([128, 2, 6], F32, tag="ms")
                nc.vector.tensor_scalar(out=ms[:], in0=ssq_ch[c][:, 2 * p:2 * p + 2, :],
                                        scalar1=0.5 / D, scalar2=EPS / 2,
                                        op0=ALU.mult, op1=ALU.add)
                rs = newton_rsqrt(ms[:].rearrange("p a b -> p (a b)"), 12, "nq")
                rsv = rs[:].rearrange("p (t k) -> p t k", t=2)
                rsq = wk.tile([128, 2, 4], F32, tag="rsq")
                nc.vector.tensor_mul(rsq[:], rsv[:, :, 0:4], lsc_sb[:, tt0:tt0 + 2, :])
                rsq_h = wk.tile([128, 2, 4], FP16, tag="rsqh", name=f"rsqh{c}_{p}")
                nc.vector.tensor_copy(out=rsq_h[:], in_=rsq[:])
                nc.vector.tensor_scalar(out=rskA[:, tt0:tt0 + 2, :], in0=rsv[:, :, 4:6],
                                        scalar1=ISQ, scalar2=None, op0=ALU.mult)
                rs_ch[(c, p)] = rsq_h

            def rope_tile(c, ti):
                tt = c * 4 + ti
                q_h, k_h = qh_ch[c][ti]
                rsq_h = rs_ch[(c, ti // 2)]
                qs = wk.tile([128, 4, 128], FP16, tag="qs")
                nc.vector.tensor_mul(qs[:], q_h[:].rearrange("p (h d) -> p h d", h=4),
                                     _bc_last(rsq_h[:, ti % 2, :], 128))
                qc = wk.tile([128, 4, 128], FP16, tag="qc")
                nc.vector.tensor_mul(qc[:], qs[:], cos_sb[:, tt])
                tq = wk.tile([128, 4, 128], FP16, tag="tq")
                nc.vector.tensor_mul(tq[:, :, 0:64], qs[:, :, 64:128], sin_sb[:, tt, :, 0:64])
                nc.vector.tensor_mul(tq[:, :, 64:128], qs[:, :, 0:64], sin_sb[:, tt, :, 64:128])
                qr = qrp.tile([128, 4, 128], FP16, tag="qr")
                nc.vector.tensor_add(qr[:], qc[:], tq[:])
                if ti == 0:
                    qT_ch[c] = qtp.tile([128, 4, TC], FP16, tag="qtc", name=f"qtc{c}")
                nc.sync.dma_start_transpose(
                    out=qT_ch[c][:, :, ti * 128:(ti + 1) * 128],
                    in_=qr[:].rearrange("p a t -> p (a t)"))
                ksv = k_h[:].rearrange("p (h d) -> p h d", h=2)
                kc = wk.tile([128, 2, 128], FP16, tag="kc")
                nc.gpsimd.tensor_mul(kc[:], ksv, cos_sb[:, tt, 0:2])
                tk = wk.tile([128, 2, 128], FP16, tag="tk")
                nc.gpsimd.tensor_mul(tk[:, :, 0:64], ksv[:, :, 64:128], sin_sb[:, tt, 0:2, 0:64])
                nc.gpsimd.tensor_mul(tk[:, :, 64:128], ksv[:, :, 0:64], sin_sb[:, tt, 0:2, 64:128])
                kr = qrp.tile([128, 2, 128], FP16, tag="kr")
                nc.gpsimd.tensor_add(kr[:], kc[:], tk[:])
                nc.sync.dma_start_transpose(
                    out=kT_t[tt][:], in_=kr[:].rearrange("p a t -> p (a t)"))

            def qkv_chunk(c):
                for p in range(2):
                    proj_tile(c, 2 * p)
                    proj_tile(c, 2 * p + 1)
                    newton_pair(c, p)
                    rope_tile(c, 2 * p)
                    rope_tile(c, 2 * p + 1)

            y1_ch = {}

            def attn_scores(c, h, s):
                pts = []
                for kt in range(4 * (c + 1)):
                    j = kt - 4 * c
                    qoff = max(j, 0) * 128
                    st = ps_s.tile([128, 512], F32, tag="st")
                    if j >= 0:
                        # preload causal mask into PSUM, accumulate scores onto it
                        nc.tensor.matmul(st[:, qoff:512], ident_sb[:],
                                         triw_sb[:, 0:512 - qoff], start=True, stop=False)
                        nc.tensor.matmul(st[:, qoff:512], kT_t[kt][:, s, :],
                                         qT_ch[c][:, 2 * s + h, qoff:512],
                                         start=False, stop=True)
                    else:
                        nc.tensor.matmul(st[:], kT_t[kt][:, s, :],
                                         qT_ch[c][:, 2 * s + h, :], start=True, stop=True)
                    pt = ptp.tile([128, 512], BF16, tag="pt")
                    nc.scalar.activation(out=pt[:, qoff:512], in_=st[:, qoff:512],
                                         func=AF.Exp, scale=rskA[:, kt, s:s + 1])
                    pts.append(pt)
                return pts

            def attn_pv(c, h, s, pts):
                if s == 0:
                    y1_ch[(c, h)] = wk.tile([128, 4, 256], F32, tag="y1", name=f"y1_{c}_{h}")
                y1 = y1_ch[(c, h)]
                yvs = []
                s2 = None
                if s == 1:
                    s2 = wk.tile([128, 4], F32, tag="s2")
                for sq in range(4):
                    qt_g = 4 * c + sq
                    o = ps_o.tile([128, 258], F32, tag="o")
                    for kt in range(qt_g + 1):
                        nc.tensor.matmul(o[:], pts[kt][:, sq * 128:(sq + 1) * 128],
                                         vA_t[kt][:], start=(kt == 0), stop=(kt == qt_g))
                    rec = wk.tile([128, 1], F32, tag="rec")
                    nc.vector.reciprocal(rec[:], o[:, 256:257])
                    if s == 0:
                        nc.vector.tensor_scalar_mul(y1[:, sq, :], o[:, 0:256], rec[:])
                    else:
                        nc.vector.tensor_scalar_mul(rec[:], rec[:], neglam_sb[:])
                        yv = yvp.tile([128, 256], F32, tag="yv")
                        nc.vector.scalar_tensor_tensor(
                            out=yv[:], in0=o[:, 0:256], scalar=rec[:],
                            in1=y1[:, sq, :], op0=ALU.mult, op1=ALU.add)
                        sq2 = wk.tile([128, 256], F32, tag="sq2")
                        nc.vector.tensor_mul(sq2[:], yv[:], yv[:])
                        nc.vector.tensor_reduce(s2[:, sq:sq + 1], sq2[:], axis=AX.X,
                                                op=ALU.add)
                        yvs.append(yv)
                if s == 1:
                    ms2 = wk.tile([128, 4], F32, tag="ms2")
                    nc.vector.tensor_scalar(out=ms2[:], in0=s2[:], scalar1=0.5 / 256,
                                            scalar2=EPS / 2, op0=ALU.mult, op1=ALU.add)
                    rsy = newton_rsqrt(ms2[:], 4, "ne", iters=1, eng=nc.vector)
                    for sq in range(4):
                        qt_g = 4 * c + sq
                        yo = yop.tile([128, 256], F32, tag="yo")
                        nc.vector.scalar_tensor_tensor(
                            out=yo[:], in0=yvs[sq][:], scalar=rsy[:, sq:sq + 1],
                            in1=gn_sb[:, h, :], op0=ALU.mult, op1=ALU.mult)
                        nc.gpsimd.dma_start(
                            out=y_d.ap()[qt_g * 128:(qt_g + 1) * 128, h, :], in_=yo[:])

            # ---------------- schedule ----------------
            HS = [(0, 0), (0, 1), (1, 0), (1, 1)]
            qkv_chunk(0)
            emit_x_load(2)
            qkv_chunk(1)
            ropeq = []    # pending (chunk, tile) rope emissions
            for c in (0, 1):
                if c == 1:
                    emit_x_load(3)
                cn = c + 2
                for i, (h, s) in enumerate(HS):
                    if ropeq:
                        rc, rt = ropeq.pop(0)
                        rope_tile(rc, rt)
                    pts = attn_scores(c, h, s)
                    proj_tile(cn, i)
                    if i % 2 == 1:
                        newton_pair(cn, i // 2)
                        ropeq.extend((cn, 2 * (i // 2) + t) for t in range(2))
                    attn_pv(c, h, s, pts)
            for i, (h, s) in enumerate(HS):
                if ropeq:
                    rc, rt = ropeq.pop(0)
                    rope_tile(rc, rt)
                pts = attn_scores(2, h, s)
                attn_pv(2, h, s, pts)
            for i, (h, s) in enumerate(HS):
                pts = attn_scores(3, h, s)
                attn_pv(3, h, s, pts)
    nc.compile()
    return nc


_NC = None
_last_in_maps = None


def _get_nc():
    global _NC
    if _NC is None:
        _NC = _build()
    return _NC


def kernel(x, Wq, Wk, Wv, lambda_q1, lambda_k1, lambda_q2, lambda_k2,
           softmax_scaler, gn_weight):
    x = np.asarray(x, np.float32)
    Wq = np.asarray(Wq, np.float32)
    Wk = np.asarray(Wk, np.float32)
    Wv = np.asarray(Wv, np.float32)
    lam = float(np.exp(np.sum(np.float64(lambda_q1) * np.float64(lambda_k1)))
                - np.exp(np.sum(np.float64(lambda_q2) * np.float64(lambda_k2)))
                + LAMBDA_INIT)
    softmax_scaler = np.asarray(softmax_scaler, np.float32)
    gn_weight = np.asarray(gn_weight, np.float32)

    nc = _get_nc()
    in_maps = []
    for core in range(8):
        b, r = divmod(core, 4)
        qheads = [2 * r, 2 * r + 1, 8 + 2 * r, 8 + 2 * r + 1]
        wq_c = np.concatenate([Wq[:, hh * 128:(hh + 1) * 128] for hh in qheads], axis=1)
        wkv_c = np.concatenate([
            Wk[:, r * 128:(r + 1) * 128],
            Wk[:, (4 + r) * 128:(5 + r) * 128],
            Wv[:, r * 256:(r + 1) * 256],
        ], axis=1)
        logp = np.log(np.arange(1, T + 1, dtype=np.float64)).reshape(NT, 128)
        lsc = (logp[:, :, None] * np.float64(softmax_scaler[qheads]).reshape(1, 1, 4))
        lsc = lsc.transpose(1, 0, 2).astype(np.float32)
        in_maps.append({
            "xT": np.ascontiguousarray(x[b].T).astype(np.float16),
            "wq": np.ascontiguousarray(wq_c).astype(np.float16),
            "wkv": np.ascontiguousarray(wkv_c).astype(np.float16),
            "lsc": np.ascontiguousarray(lsc),
            "gnw": np.ascontiguousarray(
                np.broadcast_to(gn_weight[2 * r:2 * r + 2].reshape(1, 2, 256), (128, 2, 256))).astype(np.float32),
            "neglam": np.full((128, 1), -lam, np.float32),
        })
    global _last_in_maps
    _last_in_maps = in_maps
    res = run_bass_kernel_spmd(nc, in_maps, list(range(8)))
    out = np.empty((B, T, 8, 256), np.float32)
    for core in range(8):
        b, r = divmod(core, 4)
        out[b, :, 2 * r:2 * r + 2, :] = res.results[core]["y"]
    return out


# revision 27
# speedup vs baseline: 1.1397x; 1.1397x over previous
"""MixerDiffAttention Trainium2 kernel (v3).

Sharding: 8 cores = batch(2) x head-group(4).  Core (b, r) computes output
heads {2r, 2r+1} of batch b: q-heads {2r,2r+1,8+2r,8+2r+1}, k-heads {r, 4+r},
v-head r.

Design:
 - Act engine runs ONLY Exp (zero activation-table reloads).
 - fp16 q/k pipeline (host-cast inputs), bf16 pt/vA for PV (bf16 range
   needed for max-free exp).  Scores fp16 x fp16 -> f32 PSUM.
 - Transposes via batched DMA xbar: one [128,512]->[128,4,128] per tile for
   q (sync queue) and one [128,256]->[128,2,128] for k (scalar queue).
 - Causal diag mask pre-loaded into PSUM by an extra matmul in the same
   accumulation group (no vector mask work).
 - RMS rsqrt via bit-trick + Newton on DVE; sums of squares via DVE
   mul+reduce; PSUM drains on DVE; SBUF-only elementwise (rope-k, final
   gn scale) on Pool.
 - Schedule: proj(c+2) + rope tiles interleaved into attn(c) segments so
   the PE and Act never wait on bursts.
"""
import os
import numpy as np
import concourse.bass as bass
import concourse.mybir as mybir
from concourse import bacc
from concourse.tile import TileContext
from concourse.bass_utils import run_bass_kernel_spmd

B, T, DM = 2, 2048, 2048
H, KVH, D = 16, 8, 128
TC = 512                  # token chunk (= q chunk)
NT = T // 128             # 16 token tiles
NCH = T // TC             # 4 chunks
NDM = DM // 128           # 16 contraction chunks
EPS = 1e-6
ROPE_BASE = 10000.0
LAMBDA_INIT = 0.8 - 0.6 * np.exp(-0.3 * 12)
F32 = mybir.dt.float32
FP16 = mybir.dt.float16
BF16 = mybir.dt.bfloat16
I32 = mybir.dt.int32
AF = mybir.ActivationFunctionType
ALU = mybir.AluOpType
AX = mybir.AxisListType
ISQ = float(1.0 / np.sqrt(D))
MASK_NEG = -60000.0       # fp16-safe; exp(ISQ*(s+MASK_NEG)) == 0 in f32
RSQRT_MAGIC2 = 0x5EF759DF   # seed magic for msh = ms/2 inputs


def _bc_mid(a, n):
    # [128, m] AP -> [128, n(bcast), m]
    return bass.AP(tensor=a.tensor, offset=a.offset, ap=[list(a.ap[0]), [0, n], list(a.ap[1])])


def _bc_last(a, n):
    # [128, m] AP -> [128, m, n(bcast)]
    return bass.AP(tensor=a.tensor, offset=a.offset, ap=[list(a.ap[0]), list(a.ap[1]), [0, n]])


def _build():
    nc = bacc.Bacc(None, target_bir_lowering=False)

    xT = nc.dram_tensor("xT", [DM, T], FP16, kind="ExternalInput")
    wq_d = nc.dram_tensor("wq", [DM, 512], FP16, kind="ExternalInput")
    wkv_d = nc.dram_tensor("wkv", [DM, 512], FP16, kind="ExternalInput")
    lsc_d = nc.dram_tensor("lsc", [128, NT, 4], F32, kind="ExternalInput")
    gn_d = nc.dram_tensor("gnw", [128, 2, 256], F32, kind="ExternalInput")
    neglam_d = nc.dram_tensor("neglam", [128, 1], F32, kind="ExternalInput")
    y_d = nc.dram_tensor("y", [T, 2, 256], F32, kind="ExternalOutput")

    # constant tables, laid out [128 partitions, NT tiles, ...] host-side
    pos = np.arange(T, dtype=np.float64)
    inv = ROPE_BASE ** (-np.arange(0, D, 2, dtype=np.float64) / D)  # (64,)
    ang = np.outer(pos, inv)                                       # (T, 64)
    cos_t = np.cos(ang).reshape(NT, 128, 64)
    sin_t = np.sin(ang).reshape(NT, 128, 64)
    cosf = np.concatenate([cos_t, cos_t], axis=2)          # (NT,128,128)
    sinf = np.concatenate([sin_t, -sin_t], axis=2)
    cosf4 = np.broadcast_to(cosf[:, :, None, :], (NT, 128, 4, 128))
    sinf4 = np.broadcast_to(sinf[:, :, None, :], (NT, 128, 4, 128))
    cosf4_h = cosf4.transpose(1, 0, 2, 3).astype(np.float16).copy()
    sinf4_h = sinf4.transpose(1, 0, 2, 3).astype(np.float16).copy()
    pidx = np.arange(128).reshape(128, 1)
    fidx = np.arange(512).reshape(1, 512)
    triw_h = np.where((pidx > fidx) & (fidx < 128), np.float16(MASK_NEG),
                      np.float16(0.0)).astype(np.float16)       # [128, 512]
    ident_h = np.eye(128, dtype=np.float16)

    cos_c = nc.inline_tensor(cosf4_h, "cos_c")
    sin_c = nc.inline_tensor(sinf4_h, "sin_c")
    triw_c = nc.inline_tensor(triw_h, "triw_c")
    ident_c = nc.inline_tensor(ident_h, "ident_c")

    with TileContext(nc) as tc:
        with (
            tc.tile_pool(name="wp", bufs=1) as wp,
            tc.tile_pool(name="cp", bufs=1) as cp,
            tc.tile_pool(name="xp", bufs=34) as xp,
            tc.tile_pool(name="kv", bufs=1) as kvp,
            tc.tile_pool(name="qt", bufs=2) as qtp,
            tc.tile_pool(name="wk", bufs=2) as wk,
            tc.tile_pool(name="qh", bufs=6) as qhp,
            tc.tile_pool(name="qr", bufs=4) as qrp,
            tc.tile_pool(name="pt", bufs=18) as ptp,
            tc.tile_pool(name="yv", bufs=5) as yvp,
            tc.tile_pool(name="yo", bufs=4) as yop,
            tc.tile_pool(name="ps_p", bufs=2, space="PSUM") as ps_p,
            tc.tile_pool(name="ps_s", bufs=4, space="PSUM") as ps_s,
            tc.tile_pool(name="ps_o", bufs=2, space="PSUM") as ps_o,
        ):
            # ---- persistent loads (spread across queues; wq + x(0) first) ----
            wq_sb = wp.tile([128, NDM, 512], FP16, tag="wq")
            wkv_sb = wp.tile([128, NDM, 512], FP16, tag="wkv")
            nc.sync.dma_start(out=wq_sb, in_=wq_d.ap().rearrange("(n p) m -> p n m", p=128))

            xts = {}

            def emit_x_load(c):
                tiles = []
                for dmi in range(NDM):
                    xt_t = xp.tile([128, TC], FP16, tag="xt")
                    nc.scalar.dma_start(
                        out=xt_t,
                        in_=xT.ap()[dmi * 128:(dmi + 1) * 128, c * TC:(c + 1) * TC],
                    )
                    tiles.append(xt_t)
                xts[c] = tiles

            emit_x_load(0)
            nc.sync.dma_start(out=wkv_sb, in_=wkv_d.ap().rearrange("(n p) m -> p n m", p=128))
            emit_x_load(1)

            cos_sb = cp.tile([128, NT, 4, 128], FP16, tag="cos")
            sin_sb = cp.tile([128, NT, 4, 128], FP16, tag="sin")
            lsc_sb = cp.tile([128, NT, 4], F32, tag="lsc")
            rskA = cp.tile([128, NT, 2], F32, tag="rskA")
            c15_sb = cp.tile([128, 24], F32, tag="c15")
            triw_sb = cp.tile([128, 512], FP16, tag="triw")
            ident_sb = cp.tile([128, 128], FP16, tag="ident")
            gn_sb = cp.tile([128, 2, 256], F32, tag="gn")
            neglam_sb = cp.tile([128, 1], F32, tag="neglam")
            magic_sb = cp.tile([128, 24], I32, tag="magic")
            nc.scalar.dma_start(out=cos_sb, in_=cos_c.ap())
            nc.scalar.dma_start(out=sin_sb, in_=sin_c.ap())
            nc.scalar.dma_start(out=lsc_sb, in_=lsc_d.ap())
            nc.scalar.dma_start(out=triw_sb, in_=triw_c.ap())
            nc.scalar.dma_start(out=ident_sb, in_=ident_c.ap())
            nc.scalar.dma_start(out=gn_sb, in_=gn_d.ap())
            nc.scalar.dma_start(out=neglam_sb, in_=neglam_d.ap())
            nc.vector.memset(magic_sb[:], RSQRT_MAGIC2)
            nc.vector.memset(c15_sb[:], 1.5)

            # per-token-tile persistent K^T and V(+ones)
            kT_t = [kvp.tile([128, 2, 128], FP16, tag=f"kT{i}", name=f"kT{i}") for i in range(NT)]
            vA_t = [kvp.tile([128, 258], BF16, tag=f"vA{i}", name=f"vA{i}") for i in range(NT)]
            for i in range(NT):
                nc.gpsimd.memset(vA_t[i][:, 256:258], 1.0)

            ssq_ch = {}   # [128, 4, 6] f32 per chunk
            qh_ch = {}    # list of (q_h, k_h) per chunk
            rs_ch = {}    # (rsq_h [128,4,4] fp16, rsk_h [128,4,2] fp16)
            qT_ch = {}    # [128, 4, TC] fp16 per chunk (rotating pool bufs=2)

            def proj_tile(c, ti):
                tt = c * 4 + ti
                q_ps = ps_p.tile([128, 512], F32, tag="pp")
                kv_ps = ps_p.tile([128, 512], F32, tag="pp")
                for dmi in range(NDM):
                    lhs = xts[c][dmi][:, ti * 128:(ti + 1) * 128]
                    nc.tensor.matmul(q_ps[:], lhs, wq_sb[:, dmi, :],
                                     start=(dmi == 0), stop=(dmi == NDM - 1))
                for dmi in range(NDM):
                    lhs = xts[c][dmi][:, ti * 128:(ti + 1) * 128]
                    nc.tensor.matmul(kv_ps[:], lhs, wkv_sb[:, dmi, :],
                                     start=(dmi == 0), stop=(dmi == NDM - 1))
                # drain PSUM on DVE (only engine allowed); squares via mul+reduce
                q_h = qhp.tile([128, 512], FP16, tag="qh")
                k_h = qhp.tile([128, 256], FP16, tag="kh")
                nc.vector.tensor_copy(out=q_h[:], in_=q_ps[:])
                nc.vector.tensor_copy(out=k_h[:], in_=kv_ps[:, 0:256])
                nc.vector.tensor_copy(out=vA_t[tt][:, 0:256], in_=kv_ps[:, 256:512])
                if ti == 0:
                    ssq_ch[c] = wk.tile([128, 4, 6], F32, tag="ssq", name=f"ssq{c}")
                    qh_ch[c] = []
                qh_ch[c].append((q_h, k_h))
                sqv = wk.tile([128, 4, 128], FP16, tag="sqv")
                nc.vector.tensor_mul(sqv[:], q_h[:].rearrange("p (h d) -> p h d", h=4),
                                     q_h[:].rearrange("p (h d) -> p h d", h=4))
                nc.vector.tensor_reduce(ssq_ch[c][:, ti, 0:4], sqv[:], axis=AX.X, op=ALU.add)
                skv = wk.tile([128, 2, 128], FP16, tag="skv")
                nc.vector.tensor_mul(skv[:], k_h[:].rearrange("p (h d) -> p h d", h=2),
                                     k_h[:].rearrange("p (h d) -> p h d", h=2))
                nc.vector.tensor_reduce(ssq_ch[c][:, ti, 4:6], skv[:], axis=AX.X, op=ALU.add)

            def newton_rsqrt(msh_ap, n, tag, iters=2, eng=None):
                # rsqrt(2*msh): bit-trick seed on DVE, Newton iters on Pool
                # y' = y*(1.5 - msh*y*y)
                sh = wk.tile([128, n], I32, tag=tag + "sh")
                nc.vector.tensor_scalar(out=sh[:], in0=msh_ap.bitcast(I32), scalar1=1,
                                        scalar2=None, op0=ALU.logical_shift_right)
                y0 = wk.tile([128, n], F32, tag=tag + "y0")
                nc.vector.tensor_tensor(out=y0[:].bitcast(I32), in0=magic_sb[:, 0:n],
                                        in1=sh[:], op=ALU.subtract)
                a = wk.tile([128, n], F32, tag=tag + "a")
                e = eng if eng is not None else nc.gpsimd
                for _ in range(iters):
                    e.tensor_tensor(out=a[:], in0=msh_ap, in1=y0[:], op=ALU.mult)
                    e.tensor_tensor(out=a[:], in0=a[:], in1=y0[:], op=ALU.mult)
                    e.tensor_tensor(out=a[:], in0=c15_sb[:, 0:n], in1=a[:],
                                    op=ALU.subtract)
                    e.tensor_tensor(out=y0[:], in0=y0[:], in1=a[:], op=ALU.mult)
                return y0

            def newton_pair(c, p):
                # tiles 2p, 2p+1 of chunk c: rsqrt of q/k mean-squares
                tt0 = 4 * c + 2 * p
                ms = wk.tile([128, 2, 6], F32, tag="ms")
                nc.vector.tensor_scalar(out=ms[:], in0=ssq_ch[c][:, 2 * p:2 * p + 2, :],
                                        scalar1=0.5 / D, scalar2=EPS / 2,
                                        op0=ALU.mult, op1=ALU.add)
                rs = newton_rsqrt(ms[:].rearrange("p a b -> p (a b)"), 12, "nq")
                rsv = rs[:].rearrange("p (t k) -> p t k", t=2)
                rsq = wk.tile([128, 2, 4], F32, tag="rsq")
                nc.vector.tensor_mul(rsq[:], rsv[:, :, 0:4], lsc_sb[:, tt0:tt0 + 2, :])
                rsq_h = wk.tile([128, 2, 4], FP16, tag="rsqh", name=f"rsqh{c}_{p}")
                nc.vector.tensor_copy(out=rsq_h[:], in_=rsq[:])
                nc.vector.tensor_scalar(out=rskA[:, tt0:tt0 + 2, :], in0=rsv[:, :, 4:6],
                                        scalar1=ISQ, scalar2=None, op0=ALU.mult)
                rs_ch[(c, p)] = rsq_h

            def rope_tile(c, ti):
                tt = c * 4 + ti
                q_h, k_h = qh_ch[c][ti]
                rsq_h = rs_ch[(c, ti // 2)]
                qs = wk.tile([128, 4, 128], FP16, tag="qs")
                nc.vector.tensor_mul(qs[:], q_h[:].rearrange("p (h d) -> p h d", h=4),
                                     _bc_last(rsq_h[:, ti % 2, :], 128))
                qc = wk.tile([128, 4, 128], FP16, tag="qc")
                nc.vector.tensor_mul(qc[:], qs[:], cos_sb[:, tt])
                tq = wk.tile([128, 4, 128], FP16, tag="tq")
                nc.vector.tensor_mul(tq[:, :, 0:64], qs[:, :, 64:128], sin_sb[:, tt, :, 0:64])
                nc.vector.tensor_mul(tq[:, :, 64:128], qs[:, :, 0:64], sin_sb[:, tt, :, 64:128])
                qr = qrp.tile([128, 4, 128], FP16, tag="qr")
                nc.vector.tensor_add(qr[:], qc[:], tq[:])
                if ti == 0:
                    qT_ch[c] = qtp.tile([128, 4, TC], FP16, tag="qtc", name=f"qtc{c}")
                nc.sync.dma_start_transpose(
                    out=qT_ch[c][:, :, ti * 128:(ti + 1) * 128],
                    in_=qr[:].rearrange("p a t -> p (a t)"))
                ksv = k_h[:].rearrange("p (h d) -> p h d", h=2)
                kc = wk.tile([128, 2, 128], FP16, tag="kc")
                nc.gpsimd.tensor_mul(kc[:], ksv, cos_sb[:, tt, 0:2])
                tk = wk.tile([128, 2, 128], FP16, tag="tk")
                nc.gpsimd.tensor_mul(tk[:, :, 0:64], ksv[:, :, 64:128], sin_sb[:, tt, 0:2, 0:64])
                nc.gpsimd.tensor_mul(tk[:, :, 64:128], ksv[:, :, 0:64], sin_sb[:, tt, 0:2, 64:128])
                kr = qrp.tile([128, 2, 128], FP16, tag="kr")
                nc.gpsimd.tensor_add(kr[:], kc[:], tk[:])
                nc.sync.dma_start_transpose(
                    out=kT_t[tt][:], in_=kr[:].rearrange("p a t -> p (a t)"))

            def qkv_chunk(c):
                for p in range(2):
                    proj_tile(c, 2 * p)
                    proj_tile(c, 2 * p + 1)
                    newton_pair(c, p)
                    rope_tile(c, 2 * p)
                    rope_tile(c, 2 * p + 1)

            y1_ch = {}

            def attn_scores(c, h, s):
                pts = []
                for kt in range(4 * (c + 1)):
                    j = kt - 4 * c
                    qoff = max(j, 0) * 128
                    st = ps_s.tile([128, 512], F32, tag="st")
                    if j >= 0:
                        # preload causal mask into PSUM, accumulate scores onto it
                        nc.tensor.matmul(st[:, qoff:512], ident_sb[:],
                                         triw_sb[:, 0:512 - qoff], start=True, stop=False)
                        nc.tensor.matmul(st[:, qoff:512], kT_t[kt][:, s, :],
                                         qT_ch[c][:, 2 * s + h, qoff:512],
                                         start=False, stop=True)
                    else:
                        nc.tensor.matmul(st[:], kT_t[kt][:, s, :],
                                         qT_ch[c][:, 2 * s + h, :], start=True, stop=True)
                    pt = ptp.tile([128, 512], BF16, tag="pt")
                    nc.scalar.activation(out=pt[:, qoff:512], in_=st[:, qoff:512],
                                         func=AF.Exp, scale=rskA[:, kt, s:s + 1])
                    pts.append(pt)
                return pts

            def attn_pv(c, h, s, pts):
                if s == 0:
                    y1_ch[(c, h)] = wk.tile([128, 4, 256], F32, tag="y1", name=f"y1_{c}_{h}")
                y1 = y1_ch[(c, h)]
                yvs = []
                s2 = None
                if s == 1:
                    s2 = wk.tile([128, 4], F32, tag="s2")
                for sq in range(4):
                    qt_g = 4 * c + sq
                    o = ps_o.tile([128, 258], F32, tag="o")
                    for kt in range(qt_g + 1):
                        nc.tensor.matmul(o[:], pts[kt][:, sq * 128:(sq + 1) * 128],
                                         vA_t[kt][:], start=(kt == 0), stop=(kt == qt_g))
                    rec = wk.tile([128, 1], F32, tag="rec")
                    nc.vector.reciprocal(rec[:], o[:, 256:257])
                    if s == 0:
                        nc.vector.tensor_scalar_mul(y1[:, sq, :], o[:, 0:256], rec[:])
                    else:
                        nc.vector.tensor_scalar_mul(rec[:], rec[:], neglam_sb[:])
                        yv = yvp.tile([128, 256], F32, tag="yv")
                        nc.vector.scalar_tensor_tensor(
                            out=yv[:], in0=o[:, 0:256], scalar=rec[:],
                            in1=y1[:, sq, :], op0=ALU.mult, op1=ALU.add)
                        sq2 = wk.tile([128, 256], F32, tag="sq2")
                        nc.vector.tensor_mul(sq2[:], yv[:], yv[:])
                        nc.vector.tensor_reduce(s2[:, sq:sq + 1], sq2[:], axis=AX.X,
                                                op=ALU.add)
                        yvs.append(yv)
                if s == 1:
                    ms2 = wk.tile([128, 4], F32, tag="ms2")
                    nc.vector.tensor_scalar(out=ms2[:], in0=s2[:], scalar1=0.5 / 256,
                                            scalar2=EPS / 2, op0=ALU.mult, op1=ALU.add)
                    rsy = newton_rsqrt(ms2[:], 4, "ne", iters=1, eng=nc.vector)
                    for sq in range(4):
                        qt_g = 4 * c + sq
                        yo = yop.tile([128, 256], F32, tag="yo")
                        nc.gpsimd.tensor_mul(yo[:], yvs[sq][:], gn_sb[:, h, :])
                        nc.gpsimd.tensor_mul(yo[:], yo[:], _bc_last(rsy[:, sq:sq + 1], 256))
                        nc.gpsimd.dma_start(
                            out=y_d.ap()[qt_g * 128:(qt_g + 1) * 128, h, :], in_=yo[:])

            # ---------------- schedule ----------------
            HS = [(0, 0), (0, 1), (1, 0), (1, 1)]
            qkv_chunk(0)
            emit_x_load(2)
            qkv_chunk(1)
            ropeq = []    # pending (chunk, tile) rope emissions
            for c in (0, 1):
                if c == 1:
                    emit_x_load(3)
                cn = c + 2
                for i, (h, s) in enumerate(HS):
                    if ropeq:
                        rc, rt = ropeq.pop(0)
                        rope_tile(rc, rt)
                    pts = attn_scores(c, h, s)
                    proj_tile(cn, i)
                    if i % 2 == 1:
                        newton_pair(cn, i // 2)
                        ropeq.extend((cn, 2 * (i // 2) + t) for t in range(2))
                    attn_pv(c, h, s, pts)
            for i, (h, s) in enumerate(HS):
                if ropeq:
                    rc, rt = ropeq.pop(0)
                    rope_tile(rc, rt)
                pts = attn_scores(2, h, s)
                attn_pv(2, h, s, pts)
            for i, (h, s) in enumerate(HS):
                pts = attn_scores(3, h, s)
                attn_pv(3, h, s, pts)
    nc.compile()
    return nc


_NC = None
_last_in_maps = None


def _get_nc():
    global _NC
    if _NC is None:
        _NC = _build()
    return _NC


def kernel(x, Wq, Wk, Wv, lambda_q1, lambda_k1, lambda_q2, lambda_k2,
           softmax_scaler, gn_weight):
    x = np.asarray(x, np.float32)
    Wq = np.asarray(Wq, np.float32)
    Wk = np.asarray(Wk, np.float32)
    Wv = np.asarray(Wv, np.float32)
    lam = float(np.exp(np.sum(np.float64(lambda_q1) * np.float64(lambda_k1)))
                - np.exp(np.sum(np.float64(lambda_q2) * np.float64(lambda_k2)))
                + LAMBDA_INIT)
    softmax_scaler = np.asarray(softmax_scaler, np.float32)
    gn_weight = np.asarray(gn_weight, np.float32)

    nc = _get_nc()
    in_maps = []
    for core in range(8):
        b, r = divmod(core, 4)
        qheads = [2 * r, 2 * r + 1, 8 + 2 * r, 8 + 2 * r + 1]
        wq_c = np.concatenate([Wq[:, hh * 128:(hh + 1) * 128] for hh in qheads], axis=1)
        wkv_c = np.concatenate([
            Wk[:, r * 128:(r + 1) * 128],
            Wk[:, (4 + r) * 128:(5 + r) * 128],
            Wv[:, r * 256:(r + 1) * 256],
        ], axis=1)
        logp = np.log(np.arange(1, T + 1, dtype=np.float64)).reshape(NT, 128)
        lsc = (logp[:, :, None] * np.float64(softmax_scaler[qheads]).reshape(1, 1, 4))
        lsc = lsc.transpose(1, 0, 2).astype(np.float32)
        in_maps.append({
            "xT": np.ascontiguousarray(x[b].T).astype(np.float16),
            "wq": np.ascontiguousarray(wq_c).astype(np.float16),
            "wkv": np.ascontiguousarray(wkv_c).astype(np.float16),
            "lsc": np.ascontiguousarray(lsc),
            "gnw": np.ascontiguousarray(
                np.broadcast_to(gn_weight[2 * r:2 * r + 2].reshape(1, 2, 256), (128, 2, 256))).astype(np.float32),
            "neglam": np.full((128, 1), -lam, np.float32),
        })
    global _last_in_maps
    _last_in_maps = in_maps
    res = run_bass_kernel_spmd(nc, in_maps, list(range(8)))
    out = np.empty((B, T, 8, 256), np.float32)
    for core in range(8):
        b, r = divmod(core, 4)
        out[b, :, 2 * r:2 * r + 2, :] = res.results[core]["y"]
    return out


# revision 28
# speedup vs baseline: 1.1613x; 1.0189x over previous
"""MixerDiffAttention Trainium2 kernel (v3).

Sharding: 8 cores = batch(2) x head-group(4).  Core (b, r) computes output
heads {2r, 2r+1} of batch b: q-heads {2r,2r+1,8+2r,8+2r+1}, k-heads {r, 4+r},
v-head r.

Design:
 - Act engine runs ONLY Exp (zero activation-table reloads).
 - fp16 q/k pipeline (host-cast inputs), bf16 pt/vA for PV (bf16 range
   needed for max-free exp).  Scores fp16 x fp16 -> f32 PSUM.
 - Transposes via batched DMA xbar: one [128,512]->[128,4,128] per tile for
   q (sync queue) and one [128,256]->[128,2,128] for k (scalar queue).
 - Causal diag mask pre-loaded into PSUM by an extra matmul in the same
   accumulation group (no vector mask work).
 - RMS rsqrt via bit-trick + Newton on DVE; sums of squares via DVE
   mul+reduce; PSUM drains on DVE; SBUF-only elementwise (rope-k, final
   gn scale) on Pool.
 - Schedule: proj(c+2) + rope tiles interleaved into attn(c) segments so
   the PE and Act never wait on bursts.
"""
import os
import numpy as np
import concourse.bass as bass
import concourse.mybir as mybir
from concourse import bacc
from concourse.tile import TileContext
from concourse.bass_utils import run_bass_kernel_spmd

B, T, DM = 2, 2048, 2048
H, KVH, D = 16, 8, 128
TC = 512                  # token chunk (= q chunk)
NT = T // 128             # 16 token tiles
NCH = T // TC             # 4 chunks
NDM = DM // 128           # 16 contraction chunks
EPS = 1e-6
ROPE_BASE = 10000.0
LAMBDA_INIT = 0.8 - 0.6 * np.exp(-0.3 * 12)
F32 = mybir.dt.float32
FP16 = mybir.dt.float16
BF16 = mybir.dt.bfloat16
I32 = mybir.dt.int32
AF = mybir.ActivationFunctionType
ALU = mybir.AluOpType
AX = mybir.AxisListType
ISQ = float(1.0 / np.sqrt(D))
MASK_NEG = -60000.0       # fp16-safe; exp(ISQ*(s+MASK_NEG)) == 0 in f32
RSQRT_MAGIC2 = 0x5EF759DF   # seed magic for msh = ms/2 inputs


def _bc_mid(a, n):
    # [128, m] AP -> [128, n(bcast), m]
    return bass.AP(tensor=a.tensor, offset=a.offset, ap=[list(a.ap[0]), [0, n], list(a.ap[1])])


def _bc_last(a, n):
    # [128, m] AP -> [128, m, n(bcast)]
    return bass.AP(tensor=a.tensor, offset=a.offset, ap=[list(a.ap[0]), list(a.ap[1]), [0, n]])


def _build():
    nc = bacc.Bacc(None, target_bir_lowering=False)

    xT = nc.dram_tensor("xT", [DM, T], FP16, kind="ExternalInput")
    wq_d = nc.dram_tensor("wq", [DM, 512], FP16, kind="ExternalInput")
    wkv_d = nc.dram_tensor("wkv", [DM, 512], FP16, kind="ExternalInput")
    lsc_d = nc.dram_tensor("lsc", [128, NT, 4], F32, kind="ExternalInput")
    gn_d = nc.dram_tensor("gnw", [128, 2, 256], F32, kind="ExternalInput")
    neglam_d = nc.dram_tensor("neglam", [128, 1], F32, kind="ExternalInput")
    y_d = nc.dram_tensor("y", [T, 2, 256], F32, kind="ExternalOutput")

    # constant tables, laid out [128 partitions, NT tiles, ...] host-side
    pos = np.arange(T, dtype=np.float64)
    inv = ROPE_BASE ** (-np.arange(0, D, 2, dtype=np.float64) / D)  # (64,)
    ang = np.outer(pos, inv)                                       # (T, 64)
    cos_t = np.cos(ang).reshape(NT, 128, 64)
    sin_t = np.sin(ang).reshape(NT, 128, 64)
    cosf = np.concatenate([cos_t, cos_t], axis=2)          # (NT,128,128)
    sinf = np.concatenate([sin_t, -sin_t], axis=2)
    cosf4 = np.broadcast_to(cosf[:, :, None, :], (NT, 128, 4, 128))
    sinf4 = np.broadcast_to(sinf[:, :, None, :], (NT, 128, 4, 128))
    cosf4_h = cosf4.transpose(1, 0, 2, 3).astype(np.float16).copy()
    sinf4_h = sinf4.transpose(1, 0, 2, 3).astype(np.float16).copy()
    pidx = np.arange(128).reshape(128, 1)
    fidx = np.arange(512).reshape(1, 512)
    triw_h = np.where((pidx > fidx) & (fidx < 128), np.float16(MASK_NEG),
                      np.float16(0.0)).astype(np.float16)       # [128, 512]
    ident_h = np.eye(128, dtype=np.float16)

    cos_c = nc.inline_tensor(cosf4_h, "cos_c")
    sin_c = nc.inline_tensor(sinf4_h, "sin_c")
    triw_c = nc.inline_tensor(triw_h, "triw_c")
    ident_c = nc.inline_tensor(ident_h, "ident_c")

    with TileContext(nc) as tc:
        with (
            tc.tile_pool(name="wp", bufs=1) as wp,
            tc.tile_pool(name="cp", bufs=1) as cp,
            tc.tile_pool(name="xp", bufs=34) as xp,
            tc.tile_pool(name="kv", bufs=1) as kvp,
            tc.tile_pool(name="qt", bufs=2) as qtp,
            tc.tile_pool(name="wk", bufs=2) as wk,
            tc.tile_pool(name="qh", bufs=6) as qhp,
            tc.tile_pool(name="qr", bufs=4) as qrp,
            tc.tile_pool(name="pt", bufs=18) as ptp,
            tc.tile_pool(name="yv", bufs=5) as yvp,
            tc.tile_pool(name="yo", bufs=4) as yop,
            tc.tile_pool(name="ps_p", bufs=2, space="PSUM") as ps_p,
            tc.tile_pool(name="ps_s", bufs=4, space="PSUM") as ps_s,
            tc.tile_pool(name="ps_o", bufs=2, space="PSUM") as ps_o,
        ):
            # ---- persistent loads (spread across queues; wq + x(0) first) ----
            wq_sb = wp.tile([128, NDM, 512], FP16, tag="wq")
            wkv_sb = wp.tile([128, NDM, 512], FP16, tag="wkv")
            nc.sync.dma_start(out=wq_sb, in_=wq_d.ap().rearrange("(n p) m -> p n m", p=128))

            xts = {}

            def emit_x_load(c):
                tiles = []
                for dmi in range(NDM):
                    xt_t = xp.tile([128, TC], FP16, tag="xt")
                    nc.scalar.dma_start(
                        out=xt_t,
                        in_=xT.ap()[dmi * 128:(dmi + 1) * 128, c * TC:(c + 1) * TC],
                    )
                    tiles.append(xt_t)
                xts[c] = tiles

            emit_x_load(0)
            nc.sync.dma_start(out=wkv_sb, in_=wkv_d.ap().rearrange("(n p) m -> p n m", p=128))
            emit_x_load(1)

            cos_sb = cp.tile([128, NT, 4, 128], FP16, tag="cos")
            sin_sb = cp.tile([128, NT, 4, 128], FP16, tag="sin")
            lsc_sb = cp.tile([128, NT, 4], F32, tag="lsc")
            rskA = cp.tile([128, NT, 2], F32, tag="rskA")
            c15_sb = cp.tile([128, 24], F32, tag="c15")
            triw_sb = cp.tile([128, 512], FP16, tag="triw")
            ident_sb = cp.tile([128, 128], FP16, tag="ident")
            gn_sb = cp.tile([128, 2, 256], F32, tag="gn")
            neglam_sb = cp.tile([128, 1], F32, tag="neglam")
            magic_sb = cp.tile([128, 24], I32, tag="magic")
            nc.scalar.dma_start(out=cos_sb, in_=cos_c.ap())
            nc.scalar.dma_start(out=sin_sb, in_=sin_c.ap())
            nc.scalar.dma_start(out=lsc_sb, in_=lsc_d.ap())
            nc.scalar.dma_start(out=triw_sb, in_=triw_c.ap())
            nc.scalar.dma_start(out=ident_sb, in_=ident_c.ap())
            nc.scalar.dma_start(out=gn_sb, in_=gn_d.ap())
            nc.scalar.dma_start(out=neglam_sb, in_=neglam_d.ap())
            nc.vector.memset(magic_sb[:], RSQRT_MAGIC2)
            nc.vector.memset(c15_sb[:], 1.5)

            # per-token-tile persistent K^T and V(+ones)
            kT_t = [kvp.tile([128, 2, 128], FP16, tag=f"kT{i}", name=f"kT{i}") for i in range(NT)]
            vA_t = [kvp.tile([128, 258], BF16, tag=f"vA{i}", name=f"vA{i}") for i in range(NT)]
            for i in range(NT):
                nc.gpsimd.memset(vA_t[i][:, 256:258], 1.0)

            ssq_ch = {}   # [128, 4, 6] f32 per chunk
            qh_ch = {}    # list of (q_h, k_h) per chunk
            rs_ch = {}    # (rsq_h [128,4,4] fp16, rsk_h [128,4,2] fp16)
            qT_ch = {}    # [128, 4, TC] fp16 per chunk (rotating pool bufs=2)

            def proj_tile(c, ti):
                tt = c * 4 + ti
                q_ps = ps_p.tile([128, 512], F32, tag="pp")
                kv_ps = ps_p.tile([128, 512], F32, tag="pp")
                for dmi in range(NDM):
                    lhs = xts[c][dmi][:, ti * 128:(ti + 1) * 128]
                    nc.tensor.matmul(q_ps[:], lhs, wq_sb[:, dmi, :],
                                     start=(dmi == 0), stop=(dmi == NDM - 1))
                for dmi in range(NDM):
                    lhs = xts[c][dmi][:, ti * 128:(ti + 1) * 128]
                    nc.tensor.matmul(kv_ps[:], lhs, wkv_sb[:, dmi, :],
                                     start=(dmi == 0), stop=(dmi == NDM - 1))
                # drain PSUM on DVE (only engine allowed); squares via mul+reduce
                q_h = qhp.tile([128, 512], FP16, tag="qh")
                k_h = qhp.tile([128, 256], FP16, tag="kh")
                nc.vector.tensor_copy(out=q_h[:], in_=q_ps[:])
                nc.vector.tensor_copy(out=k_h[:], in_=kv_ps[:, 0:256])
                nc.vector.tensor_copy(out=vA_t[tt][:, 0:256], in_=kv_ps[:, 256:512])
                if ti == 0:
                    ssq_ch[c] = wk.tile([128, 4, 6], F32, tag="ssq", name=f"ssq{c}")
                    qh_ch[c] = []
                qh_ch[c].append((q_h, k_h))
                sqv = wk.tile([128, 4, 128], FP16, tag="sqv")
                nc.vector.tensor_mul(sqv[:], q_h[:].rearrange("p (h d) -> p h d", h=4),
                                     q_h[:].rearrange("p (h d) -> p h d", h=4))
                nc.vector.tensor_reduce(ssq_ch[c][:, ti, 0:4], sqv[:], axis=AX.X, op=ALU.add)
                skv = wk.tile([128, 2, 128], FP16, tag="skv")
                nc.vector.tensor_mul(skv[:], k_h[:].rearrange("p (h d) -> p h d", h=2),
                                     k_h[:].rearrange("p (h d) -> p h d", h=2))
                nc.vector.tensor_reduce(ssq_ch[c][:, ti, 4:6], skv[:], axis=AX.X, op=ALU.add)

            def newton_rsqrt(msh_ap, n, tag, iters=2, eng=None):
                # rsqrt(2*msh): bit-trick seed on DVE, Newton iters on Pool
                # y' = y*(1.5 - msh*y*y)
                sh = wk.tile([128, n], I32, tag=tag + "sh")
                nc.vector.tensor_scalar(out=sh[:], in0=msh_ap.bitcast(I32), scalar1=1,
                                        scalar2=None, op0=ALU.logical_shift_right)
                y0 = wk.tile([128, n], F32, tag=tag + "y0")
                nc.vector.tensor_tensor(out=y0[:].bitcast(I32), in0=magic_sb[:, 0:n],
                                        in1=sh[:], op=ALU.subtract)
                a = wk.tile([128, n], F32, tag=tag + "a")
                e = eng if eng is not None else nc.gpsimd
                for _ in range(iters):
                    e.tensor_tensor(out=a[:], in0=msh_ap, in1=y0[:], op=ALU.mult)
                    e.tensor_tensor(out=a[:], in0=a[:], in1=y0[:], op=ALU.mult)
                    e.tensor_tensor(out=a[:], in0=c15_sb[:, 0:n], in1=a[:],
                                    op=ALU.subtract)
                    e.tensor_tensor(out=y0[:], in0=y0[:], in1=a[:], op=ALU.mult)
                return y0

            def newton_pair(c, p):
                # tiles 2p, 2p+1 of chunk c: rsqrt of q/k mean-squares
                tt0 = 4 * c + 2 * p
                ms = wk.tile([128, 2, 6], F32, tag="ms")
                nc.vector.tensor_scalar(out=ms[:], in0=ssq_ch[c][:, 2 * p:2 * p + 2, :],
                                        scalar1=0.5 / D, scalar2=EPS / 2,
                                        op0=ALU.mult, op1=ALU.add)
                rs = newton_rsqrt(ms[:].rearrange("p a b -> p (a b)"), 12, "nq")
                rsv = rs[:].rearrange("p (t k) -> p t k", t=2)
                rsq = wk.tile([128, 2, 4], F32, tag="rsq")
                nc.vector.tensor_mul(rsq[:], rsv[:, :, 0:4], lsc_sb[:, tt0:tt0 + 2, :])
                rsq_h = wk.tile([128, 2, 4], FP16, tag="rsqh", name=f"rsqh{c}_{p}")
                nc.vector.tensor_copy(out=rsq_h[:], in_=rsq[:])
                nc.vector.tensor_scalar(out=rskA[:, tt0:tt0 + 2, :], in0=rsv[:, :, 4:6],
                                        scalar1=ISQ, scalar2=None, op0=ALU.mult)
                rs_ch[(c, p)] = rsq_h

            def rope_tile(c, ti):
                tt = c * 4 + ti
                q_h, k_h = qh_ch[c][ti]
                rsq_h = rs_ch[(c, ti // 2)]
                qs = wk.tile([128, 4, 128], FP16, tag="qs")
                nc.vector.tensor_mul(qs[:], q_h[:].rearrange("p (h d) -> p h d", h=4),
                                     _bc_last(rsq_h[:, ti % 2, :], 128))
                qc = wk.tile([128, 4, 128], FP16, tag="qc")
                nc.vector.tensor_mul(qc[:], qs[:], cos_sb[:, tt])
                tq = wk.tile([128, 4, 128], FP16, tag="tq")
                nc.vector.tensor_mul(tq[:, :, 0:64], qs[:, :, 64:128], sin_sb[:, tt, :, 0:64])
                nc.vector.tensor_mul(tq[:, :, 64:128], qs[:, :, 0:64], sin_sb[:, tt, :, 64:128])
                qr = qrp.tile([128, 4, 128], FP16, tag="qr")
                nc.vector.tensor_add(qr[:], qc[:], tq[:])
                if ti == 0:
                    qT_ch[c] = qtp.tile([128, 4, TC], FP16, tag="qtc", name=f"qtc{c}")
                nc.sync.dma_start_transpose(
                    out=qT_ch[c][:, :, ti * 128:(ti + 1) * 128],
                    in_=qr[:].rearrange("p a t -> p (a t)"))
                ksv = k_h[:].rearrange("p (h d) -> p h d", h=2)
                kc = wk.tile([128, 2, 128], FP16, tag="kc")
                nc.gpsimd.tensor_mul(kc[:], ksv, cos_sb[:, tt, 0:2])
                tk = wk.tile([128, 2, 128], FP16, tag="tk")
                nc.gpsimd.tensor_mul(tk[:, :, 0:64], ksv[:, :, 64:128], sin_sb[:, tt, 0:2, 0:64])
                nc.gpsimd.tensor_mul(tk[:, :, 64:128], ksv[:, :, 0:64], sin_sb[:, tt, 0:2, 64:128])
                kr = qrp.tile([128, 2, 128], FP16, tag="kr")
                nc.gpsimd.tensor_add(kr[:], kc[:], tk[:])
                nc.sync.dma_start_transpose(
                    out=kT_t[tt][:], in_=kr[:].rearrange("p a t -> p (a t)"))

            def qkv_chunk(c):
                for p in range(2):
                    proj_tile(c, 2 * p)
                    proj_tile(c, 2 * p + 1)
                    newton_pair(c, p)
                    rope_tile(c, 2 * p)
                    rope_tile(c, 2 * p + 1)

            y1_ch = {}

            def attn_scores(c, h, s, borrow=False):
                pts = []
                for kt in range(4 * (c + 1)):
                    j = kt - 4 * c
                    qoff = max(j, 0) * 128
                    if borrow and kt % 3 == 2:
                        st = ps_p.tile([128, 512], F32, tag="pp")
                    else:
                        st = ps_s.tile([128, 512], F32, tag="st")
                    if j >= 0:
                        # preload causal mask into PSUM, accumulate scores onto it
                        nc.tensor.matmul(st[:, qoff:512], ident_sb[:],
                                         triw_sb[:, 0:512 - qoff], start=True, stop=False)
                        nc.tensor.matmul(st[:, qoff:512], kT_t[kt][:, s, :],
                                         qT_ch[c][:, 2 * s + h, qoff:512],
                                         start=False, stop=True)
                    else:
                        nc.tensor.matmul(st[:], kT_t[kt][:, s, :],
                                         qT_ch[c][:, 2 * s + h, :], start=True, stop=True)
                    pt = ptp.tile([128, 512], BF16, tag="pt")
                    nc.scalar.activation(out=pt[:, qoff:512], in_=st[:, qoff:512],
                                         func=AF.Exp, scale=rskA[:, kt, s:s + 1])
                    pts.append(pt)
                return pts

            def attn_pv(c, h, s, pts):
                if s == 0:
                    y1_ch[(c, h)] = wk.tile([128, 4, 256], F32, tag="y1", name=f"y1_{c}_{h}")
                y1 = y1_ch[(c, h)]
                yvs = []
                s2 = None
                if s == 1:
                    s2 = wk.tile([128, 4], F32, tag="s2")
                for sq in range(4):
                    qt_g = 4 * c + sq
                    o = ps_o.tile([128, 258], F32, tag="o")
                    for kt in range(qt_g + 1):
                        nc.tensor.matmul(o[:], pts[kt][:, sq * 128:(sq + 1) * 128],
                                         vA_t[kt][:], start=(kt == 0), stop=(kt == qt_g))
                    rec = wk.tile([128, 1], F32, tag="rec")
                    nc.vector.reciprocal(rec[:], o[:, 256:257])
                    if s == 0:
                        nc.vector.tensor_scalar_mul(y1[:, sq, :], o[:, 0:256], rec[:])
                    else:
                        nc.vector.tensor_scalar_mul(rec[:], rec[:], neglam_sb[:])
                        yv = yvp.tile([128, 256], F32, tag="yv")
                        nc.vector.scalar_tensor_tensor(
                            out=yv[:], in0=o[:, 0:256], scalar=rec[:],
                            in1=y1[:, sq, :], op0=ALU.mult, op1=ALU.add)
                        sq2 = wk.tile([128, 256], F32, tag="sq2")
                        nc.vector.tensor_mul(sq2[:], yv[:], yv[:])
                        nc.vector.tensor_reduce(s2[:, sq:sq + 1], sq2[:], axis=AX.X,
                                                op=ALU.add)
                        yvs.append(yv)
                if s == 1:
                    ms2 = wk.tile([128, 4], F32, tag="ms2")
                    nc.vector.tensor_scalar(out=ms2[:], in0=s2[:], scalar1=0.5 / 256,
                                            scalar2=EPS / 2, op0=ALU.mult, op1=ALU.add)
                    rsy = newton_rsqrt(ms2[:], 4, "ne", iters=1, eng=nc.vector)
                    for sq in range(4):
                        qt_g = 4 * c + sq
                        yo = yop.tile([128, 256], F32, tag="yo")
                        nc.gpsimd.tensor_mul(yo[:], yvs[sq][:], gn_sb[:, h, :])
                        nc.gpsimd.tensor_mul(yo[:], yo[:], _bc_last(rsy[:, sq:sq + 1], 256))
                        nc.gpsimd.dma_start(
                            out=y_d.ap()[qt_g * 128:(qt_g + 1) * 128, h, :], in_=yo[:])

            # ---------------- schedule ----------------
            HS = [(0, 0), (0, 1), (1, 0), (1, 1)]
            qkv_chunk(0)
            emit_x_load(2)
            qkv_chunk(1)
            ropeq = []    # pending (chunk, tile) rope emissions
            for c in (0, 1):
                if c == 1:
                    emit_x_load(3)
                cn = c + 2
                for i, (h, s) in enumerate(HS):
                    if ropeq:
                        rc, rt = ropeq.pop(0)
                        rope_tile(rc, rt)
                    pts = attn_scores(c, h, s)
                    proj_tile(cn, i)
                    if i % 2 == 1:
                        newton_pair(cn, i // 2)
                        ropeq.extend((cn, 2 * (i // 2) + t) for t in range(2))
                    attn_pv(c, h, s, pts)
            for i, (h, s) in enumerate(HS):
                if ropeq:
                    rc, rt = ropeq.pop(0)
                    rope_tile(rc, rt)
                pts = attn_scores(2, h, s, borrow=True)
                attn_pv(2, h, s, pts)
            for i, (h, s) in enumerate(HS):
                pts = attn_scores(3, h, s, borrow=True)
                attn_pv(3, h, s, pts)
    nc.compile()
    return nc


_NC = None
_last_in_maps = None


def _get_nc():
    global _NC
    if _NC is None:
        _NC = _build()
    return _NC


def kernel(x, Wq, Wk, Wv, lambda_q1, lambda_k1, lambda_q2, lambda_k2,
           softmax_scaler, gn_weight):
    x = np.asarray(x, np.float32)
    Wq = np.asarray(Wq, np.float32)
    Wk = np.asarray(Wk, np.float32)
    Wv = np.asarray(Wv, np.float32)
    lam = float(np.exp(np.sum(np.float64(lambda_q1) * np.float64(lambda_k1)))
                - np.exp(np.sum(np.float64(lambda_q2) * np.float64(lambda_k2)))
                + LAMBDA_INIT)
    softmax_scaler = np.asarray(softmax_scaler, np.float32)
    gn_weight = np.asarray(gn_weight, np.float32)

    nc = _get_nc()
    in_maps = []
    for core in range(8):
        b, r = divmod(core, 4)
        qheads = [2 * r, 2 * r + 1, 8 + 2 * r, 8 + 2 * r + 1]
        wq_c = np.concatenate([Wq[:, hh * 128:(hh + 1) * 128] for hh in qheads], axis=1)
        wkv_c = np.concatenate([
            Wk[:, r * 128:(r + 1) * 128],
            Wk[:, (4 + r) * 128:(5 + r) * 128],
            Wv[:, r * 256:(r + 1) * 256],
        ], axis=1)
        logp = np.log(np.arange(1, T + 1, dtype=np.float64)).reshape(NT, 128)
        lsc = (logp[:, :, None] * np.float64(softmax_scaler[qheads]).reshape(1, 1, 4))
        lsc = lsc.transpose(1, 0, 2).astype(np.float32)
        in_maps.append({
            "xT": np.ascontiguousarray(x[b].T).astype(np.float16),
            "wq": np.ascontiguousarray(wq_c).astype(np.float16),
            "wkv": np.ascontiguousarray(wkv_c).astype(np.float16),
            "lsc": np.ascontiguousarray(lsc),
            "gnw": np.ascontiguousarray(
                np.broadcast_to(gn_weight[2 * r:2 * r + 2].reshape(1, 2, 256), (128, 2, 256))).astype(np.float32),
            "neglam": np.full((128, 1), -lam, np.float32),
        })
    global _last_in_maps
    _last_in_maps = in_maps
    res = run_bass_kernel_spmd(nc, in_maps, list(range(8)))
    out = np.empty((B, T, 8, 256), np.float32)
    for core in range(8):
        b, r = divmod(core, 4)
        out[b, :, 2 * r:2 * r + 2, :] = res.results[core]["y"]
    return out
